# revision 7
# baseline (speedup 1.0000x reference)
"""Trainium2 Bass kernel for nn_DA_conv: per-sample generated depthwise 3x3 conv
-> relu -> 1x1 pointwise conv (+bias) -> + x * channel_attention(altitude).

Data-parallel over batch: 8 samples -> 8 NeuronCores, weights replicated.

v5 design (trace-driven, from the 81.8us v4):
  * Depthwise three-way split per 32-row chunk: TensorE rows [0,19) (diag bf16
    matmuls), VectorE rows [19,32) (tensor_scalar_mul 4x + depth-4
    tensor_tensor add tree 2x), ScalarE computes 2 of the 9 tap products
    (activation Copy with per-partition scale) on chunks 1-3.
    Measured: TS = 208+0.266N, TT = 153+0.518N, MM N=512 gap 218ns.
  * Residual x*att rides the pointwise PSUM group as a diag(att) matmul.
  * ~16 dummy warm-up matmuls on a zeroed tile right after the preamble
    barrier: keeps the PE HAM clock at 2.4GHz before the first real taps
    (PE is otherwise idle for ~6us waiting on weight DMAs and runs cold).
  * DMA order tuned for the ~2us per-DMA completion latency: weight blob,
    mask half 1 (diag build), chunk-0 x rows [0,18), mask half 2, rest.
  * Output bf16 (host upcasts). ~8us fixed framework postamble remains.
"""

import os
from collections import deque
from contextlib import ExitStack

import ml_dtypes
import numpy as np

import concourse.bass as bass
import concourse.mybir as mybir
import concourse.tile as tile
from concourse import bacc
from concourse.bass_utils import run_bass_kernel_spmd

AF = mybir.ActivationFunctionType
ALU = mybir.AluOpType
F32 = mybir.dt.float32
BF16 = mybir.dt.bfloat16

B, C, H, W = 8, 128, 128, 128
KK = 3
NT = KK * KK                 # 9 taps
HW = H * W
XOFF = 2                     # interior column offset in the padded layout
WP = W + 4                   # host-padded width (2 left, 2 right)
HP = H + 2                   # host-padded height (1 halo row each side)
R = 32                       # image rows per chunk
NCH = H // R                 # 4 chunks
PE_ROWS = 19                 # chunk rows [0, PE_ROWS) -> TensorE depthwise
DVE_ROWS = R - PE_ROWS       # chunk rows [PE_ROWS, R) -> Vector/Scalar E
ACT_TAPS = 2                 # tap products computed on ScalarE (chunks >= 1)
TAPS = [(dy, dx) for dy in (-1, 0, 1) for dx in (-1, 0, 1)]  # t = (dy+1)*3+(dx+1)
TAIL_LAG = 2                 # 8-row tail units kept pending (pipelining depth)
N_WARM = 16                  # PE warm-up matmuls

# bf16 weight blob column layout: w1t | alt | ca1t | ca2t | w2t | cwt
W_W1T, W_ALT, W_CA1T, W_CA2T = 0, 128, 129, 145
W_W2T, W_CWT = 145 + 128, 145 + 128 + NT * 128
W_COLS = W_CWT + 128
MASK_SPLIT = 5 * 128         # mask columns for the first diag-build half

last_results = None          # BassKernelResults of the most recent run


def _pe_blocks():
    blocks = []
    r = 0
    while r < PE_ROWS:
        rr = min(4, PE_ROWS - r)
        blocks.append((r, r + rr))
        r += rr
    return blocks


def _emit(tc, nc, d):
    ctx = d["ctx"]
    singles = ctx.enter_context(tc.tile_pool(name="singles", bufs=1))
    xpool = ctx.enter_context(tc.tile_pool(name="xpool", bufs=3))
    xbpool = ctx.enter_context(tc.tile_pool(name="xbpool", bufs=2))
    spool = ctx.enter_context(tc.tile_pool(name="spool", bufs=2))
    tpool = ctx.enter_context(tc.tile_pool(name="tpool", bufs=12))
    opool = ctx.enter_context(tc.tile_pool(name="opool", bufs=3))
    pss_pool = ctx.enter_context(tc.tile_pool(name="psum_s", bufs=5, space="PSUM"))
    pso_pool = ctx.enter_context(tc.tile_pool(name="psum_o", bufs=3, space="PSUM"))

    # -- PE warm-up: dummy matmuls on a zeroed tile, no DMA dependency --
    warm = singles.tile([128, 512], BF16, name="warm", tag="warm")
    nc.gpsimd.memset(warm, 0.0)
    wps = pso_pool.tile([128, 512], F32, name="wps", tag="pso")
    for _ in range(N_WARM):
        nc.tensor.matmul(wps, lhsT=warm[:, 0:128], rhs=warm, start=True, stop=True)

    wblob = singles.tile([128, W_COLS], BF16, name="wblob", tag="wblob")
    nc.sync.dma_start(out=wblob, in_=d["wblob"])
    mask = singles.tile([128, NT * 128], BF16, name="mask", tag="mask")
    nc.sync.dma_start(out=mask[:, 0:MASK_SPLIT], in_=d["mask"][:, 0:MASK_SPLIT])

    x3 = d["xpad"].rearrange("c (h w) -> c h w", w=WP)
    xpf_d = d["xpad"]
    NB1 = (DVE_ROWS + 2) * WP    # xb1 flat length (DVE tap rows + dy halo)

    xps, xb1s = [], []
    xp0 = xpool.tile([128, R + 2, WP], BF16, name="xp0", tag="xp")
    nc.sync.dma_start(out=xp0[:, 0:18, :], in_=x3[:, 0:18, :])
    nc.sync.dma_start(
        out=mask[:, MASK_SPLIT : NT * 128], in_=d["mask"][:, MASK_SPLIT : NT * 128]
    )
    cb = singles.tile([128, 1], F32, name="cb", tag="cb")
    nc.sync.dma_start(out=cb, in_=d["cb"])
    nc.sync.dma_start(out=xp0[:, 18 : R + 2, :], in_=x3[:, 18 : R + 2, :])
    xb1_0 = xbpool.tile([128, NB1], BF16, name="xb1_0", tag="xb1")
    nc.sync.dma_start(
        out=xb1_0[:, 0 : NB1 - 2],
        in_=xpf_d[:, PE_ROWS * WP + 1 : PE_ROWS * WP + NB1 - 1],
    )
    xps.append(xp0)
    xb1s.append(xb1_0)
    _emit_prologue(tc, nc, d, singles, pss_pool, wblob, mask)
    for ci in range(1, NCH):
        y0 = ci * R
        xp = xpool.tile([128, R + 2, WP], BF16, name=f"xp{ci}", tag="xp")
        nc.sync.dma_start(out=xp, in_=x3[:, y0 : y0 + R + 2, :])
        xb1 = xbpool.tile([128, NB1], BF16, name=f"xb1{ci}", tag="xb1")
        base = (y0 + PE_ROWS) * WP
        nc.sync.dma_start(
            out=xb1[:, 0 : NB1 - 2], in_=xpf_d[:, base + 1 : base + NB1 - 1]
        )
        xps.append(xp)
        xb1s.append(xb1)

    ktab = d["ktab"]
    dg_all = d["dg_all"]
    attd = d["attd"]
    cwt = wblob[:, W_CWT : W_CWT + 128]
    out_d = d["out"]

    tails = deque()

    def flush(n):
        while len(tails) > n:
            tails.popleft()()

    for ci in range(NCH):
        y0 = ci * R
        xp = xps[ci]
        xb13 = xb1s[ci].rearrange("p (r c) -> p r c", c=WP)
        srelu = spool.tile([128, R * W], BF16, name=f"sr{ci}", tag="sr")

        # -- DVE/ACT depthwise: rows [PE_ROWS, R): 9 products, add tree --
        n_act = ACT_TAPS if ci >= 1 else 0
        prods = []
        for ti, (dy, dx) in enumerate(TAPS):
            if dx == 0:
                src = xp[:, 1 + PE_ROWS + dy : 1 + PE_ROWS + DVE_ROWS + dy,
                         XOFF : XOFF + W]
            elif dx == 1:
                src = xb13[:, 1 + dy : 1 + DVE_ROWS + dy, XOFF : XOFF + W]
            else:
                src = xb13[:, 1 + dy : 1 + DVE_ROWS + dy, 0:W]
            t = tpool.tile([128, DVE_ROWS * W], BF16, name=f"tp{ci}_{ti}", tag="tp")
            if ti < n_act:
                nc.scalar.activation(t, src, AF.Copy, scale=ktab[:, ti : ti + 1])
            else:
                nc.vector.tensor_scalar_mul(
                    out=t, in0=src, scalar1=ktab[:, ti : ti + 1]
                )
            prods.append(t)
        while len(prods) > 1:
            nxt = []
            for i in range(0, len(prods) - 1, 2):
                a, b = prods[i], prods[i + 1]
                nc.vector.tensor_tensor(out=a, in0=b, in1=a, op=ALU.add)
                nxt.append(a)
            if len(prods) % 2:
                nxt.append(prods[-1])
            prods = nxt
        nc.scalar.activation(srelu[:, PE_ROWS * W : R * W], prods[0], AF.Relu)

        # -- PE depthwise: rows [0, PE_ROWS) in <=4-row PSUM blocks --
        for rs, re in _pe_blocks():
            rows = re - rs
            pss = pss_pool.tile([128, rows * W], F32, name=f"pss{ci}_{rs}",
                                tag="pss")
            for ti, (dy, dx) in enumerate(TAPS):
                rhs = xp[:, 1 + rs + dy : 1 + re + dy, XOFF + dx : XOFF + dx + W]
                nc.tensor.matmul(
                    pss, lhsT=dg_all[:, ti * 128 : (ti + 1) * 128], rhs=rhs,
                    start=(ti == 0), stop=(ti == NT - 1),
                )
            nc.scalar.activation(srelu[:, rs * W : re * W], pss, AF.Relu)

        # -- tails: 8-row units (2 PSUM banks of pointwise+residual, 1 store) --
        last = ci == NCH - 1
        for tr in range(0, R, 8):
            tails.append(_make_tail(nc, pso_pool, opool, xp, srelu, cwt, attd,
                                    cb, out_d, ci, tr, y0, split_store=last))
            flush(0 if last else TAIL_LAG)
    flush(0)


def _emit_prologue(tc, nc, d, singles, pss_pool, wblob, mask):
    alt = wblob[:, W_ALT : W_ALT + 1]
    w1t = wblob[:, W_W1T : W_W1T + 128]
    ca1t = wblob[:, W_CA1T : W_CA1T + 16]
    ca2t = wblob[0:16, W_CA2T : W_CA2T + 128]
    w2t = wblob[:, W_W2T : W_W2T + NT * 128]

    def leaky(name, psum_src, parts, dt=F32):
        tmp = singles.tile([parts, 1], F32, name=f"{name}_t", tag=f"{name}_t")
        nc.scalar.activation(tmp, psum_src, AF.Copy)
        res = singles.tile([parts, 1], dt, name=name, tag=name)
        nc.vector.scalar_tensor_tensor(
            out=res, in0=tmp, scalar=0.1, in1=tmp, op0=ALU.mult, op1=ALU.max
        )
        return res

    # ---- kernel-generator MLP (all bf16 matmuls) ----
    feat_ps = pss_pool.tile([128, 1], F32, name="feat_ps", tag="pss")
    nc.tensor.matmul(feat_ps, lhsT=w1t, rhs=alt, start=True, stop=True)
    feat = leaky("feat", feat_ps, 128, dt=BF16)

    ktab_ps = pss_pool.tile([128, NT], F32, name="ktab_ps", tag="pss")
    w2r = w2t.rearrange("p (c t) -> p t c", t=NT)
    for t in range(NT):
        nc.tensor.matmul(
            ktab_ps[:, t : t + 1], lhsT=w2r[:, t, :], rhs=feat, start=True, stop=True
        )
    # SBUF copy (DVE/ACT tap scalar source) runs in parallel with the dg build.
    ktab = singles.tile([128, NT], F32, name="ktab", tag="ktab")
    nc.scalar.activation(ktab, ktab_ps, AF.Copy)

    # ---- diag weights straight from PSUM: dg[:, t*128+j] = I[p,j]*ktab[p,t]
    dg_all = singles.tile([128, NT * 128], BF16, name="dg_all", tag="dg_all")
    ktab_b = ktab_ps.unsqueeze(2).broadcast_to([128, NT, 128])
    mask3 = mask.rearrange("p (t c) -> p t c", t=NT)
    dg3 = dg_all.rearrange("p (t c) -> p t c", t=NT)
    nc.vector.tensor_tensor(
        out=dg3[:, 0:5, :], in0=mask3[:, 0:5, :], in1=ktab_b[:, 0:5, :],
        op=ALU.mult,
    )
    nc.vector.tensor_tensor(
        out=dg3[:, 5:NT, :], in0=mask3[:, 5:NT, :], in1=ktab_b[:, 5:NT, :],
        op=ALU.mult,
    )

    # ---- channel attention (bf16 matmuls) ----
    a1_ps = pss_pool.tile([16, 1], F32, name="a1_ps", tag="pss")
    nc.tensor.matmul(a1_ps, lhsT=ca1t, rhs=alt, start=True, stop=True)
    a1 = leaky("a1", a1_ps, 16, dt=BF16)
    att_ps = pss_pool.tile([128, 1], F32, name="att_ps", tag="pss")
    nc.tensor.matmul(att_ps, lhsT=ca2t, rhs=a1, start=True, stop=True)
    attv = singles.tile([128, 1], F32, name="attv", tag="attv")
    nc.scalar.activation(attv, att_ps, AF.Sigmoid)
    attd = singles.tile([128, 128], BF16, name="attd", tag="attd")
    nc.vector.tensor_tensor(
        out=attd, in0=mask[:, 0:128], in1=attv.broadcast_to([128, 128]),
        op=ALU.mult,
    )
    d["ktab"] = ktab
    d["dg_all"] = dg_all
    d["attd"] = attd


def _make_tail(nc, pso_pool, opool, xp, srelu, cwt, attd, cb, out_d, ci, tr, y0,
               split_store=False):
    """Two 4-row pointwise+residual PSUM groups, biased bf16 evacs into one
    8-row osb. One 8-row store, or two 4-row stores when draining the end."""

    def tail():
        osb = opool.tile([128, 8 * W], BF16, name=f"ob{ci}_{tr}", tag="ob")
        for h, r0 in enumerate((tr, tr + 4)):
            sl = slice(r0 * W, (r0 + 4) * W)
            pso = pso_pool.tile([128, 4 * W], F32, name=f"pso{ci}_{r0}", tag="pso")
            nc.tensor.matmul(pso, lhsT=cwt, rhs=srelu[:, sl], start=True,
                             stop=False)
            nc.tensor.matmul(
                pso, lhsT=attd, rhs=xp[:, 1 + r0 : 1 + r0 + 4, XOFF : XOFF + W],
                start=False, stop=True,
            )
            nc.scalar.activation(osb[:, h * 4 * W : (h + 1) * 4 * W], pso,
                                 AF.Identity, bias=cb)
            if split_store:
                nc.sync.dma_start(
                    out=out_d[:, (y0 + r0) * W : (y0 + r0 + 4) * W],
                    in_=osb[:, h * 4 * W : (h + 1) * 4 * W],
                )
        if not split_store:
            nc.sync.dma_start(
                out=out_d[:, (y0 + tr) * W : (y0 + tr + 8) * W], in_=osb
            )

    return tail


def build_module():
    nc = bacc.Bacc(
        "TRN2",
        target_bir_lowering=False,
        debug=False,
        enable_asserts=False,
        num_devices=B,
    )
    d = {
        "xpad": nc.dram_tensor("xpad", [C, HP * WP], BF16, kind="ExternalInput").ap(),
        "wblob": nc.dram_tensor("wblob", [128, W_COLS], BF16, kind="ExternalInput").ap(),
        "cb": nc.dram_tensor("cb", [C, 1], F32, kind="ExternalInput").ap(),
        "mask": nc.dram_tensor("mask", [128, NT * 128], BF16, kind="ExternalInput").ap(),
        "out": nc.dram_tensor("out", [C, HW], BF16, kind="ExternalOutput").ap(),
    }
    with tile.TileContext(nc) as tc:
        with ExitStack() as ctx:
            d["ctx"] = ctx
            _emit(tc, nc, d)
    nc.finalize()
    return nc


_module_cache = None


def _get_module():
    global _module_cache
    if _module_cache is None:
        _module_cache = build_module()
    return _module_cache


def make_in_maps(x, altitude, W1, W2, conv_w, conv_b, ca_w1, ca_w2):
    f = np.float32
    bf = ml_dtypes.bfloat16
    x = np.asarray(x, dtype=f)
    altitude = np.asarray(altitude, dtype=f)
    xpad = np.zeros((B, C, HP, WP), dtype=f)
    xpad[:, :, 1 : H + 1, XOFF : XOFF + W] = x
    xq = np.ascontiguousarray(xpad.astype(bf).reshape(B, C, HP * WP))

    wblob_shared = np.zeros((128, W_COLS), dtype=bf)
    wblob_shared[:, W_W1T : W_W1T + 128] = np.asarray(W1, dtype=f).T.astype(bf)
    wblob_shared[:, W_CA1T : W_CA1T + 16] = np.asarray(ca_w1, dtype=f).T.astype(bf)
    wblob_shared[0:16, W_CA2T : W_CA2T + 128] = np.asarray(
        ca_w2, dtype=f
    ).T.astype(bf)
    wblob_shared[:, W_W2T : W_W2T + NT * 128] = np.asarray(
        W2, dtype=f
    ).T.astype(bf)
    wblob_shared[:, W_CWT : W_CWT + 128] = np.asarray(conv_w, dtype=f).T.astype(bf)

    cb_arr = np.ascontiguousarray(np.asarray(conv_b, dtype=f).reshape(C, 1))
    mask_arr = np.ascontiguousarray(
        np.tile(np.eye(128, dtype=f), (1, NT)).astype(bf)
    )

    maps = []
    for bb in range(B):
        wblob = wblob_shared.copy()
        wblob[:, W_ALT] = altitude[bb].astype(bf)
        maps.append({"xpad": xq[bb], "wblob": np.ascontiguousarray(wblob),
                     "cb": cb_arr, "mask": mask_arr})
    return maps


def kernel(x, altitude, W1, W2, conv_w, conv_b, ca_w1, ca_w2):
    global last_results
    in_maps = make_in_maps(x, altitude, W1, W2, conv_w, conv_b, ca_w1, ca_w2)
    nc = _get_module()
    trace = os.environ.get("KERNEL_TRACE", "0") == "1"
    last_results = run_bass_kernel_spmd(
        nc, in_maps, core_ids=list(range(B)), trace=trace
    )
    out = np.stack(
        [
            last_results.results[bb]["out"].astype(np.float32).reshape(C, H, W)
            for bb in range(B)
        ]
    )
    return out


# revision 8
# speedup vs baseline: 1.0610x; 1.0610x over previous
"""Trainium2 Bass kernel for nn_DA_conv: per-sample generated depthwise 3x3 conv
-> relu -> 1x1 pointwise conv (+bias) -> + x * channel_attention(altitude).

Data-parallel over batch: 8 samples -> 8 NeuronCores, weights replicated.

v5 design (trace-driven, from the 81.8us v4):
  * Depthwise three-way split per 32-row chunk: TensorE rows [0,19) (diag bf16
    matmuls), VectorE rows [19,32) (tensor_scalar_mul 4x + depth-4
    tensor_tensor add tree 2x), ScalarE computes 2 of the 9 tap products
    (activation Copy with per-partition scale) on chunks 1-3.
    Measured: TS = 208+0.266N, TT = 153+0.518N, MM N=512 gap 218ns.
  * Residual x*att rides the pointwise PSUM group as a diag(att) matmul.
  * ~16 dummy warm-up matmuls on a zeroed tile right after the preamble
    barrier: keeps the PE HAM clock at 2.4GHz before the first real taps
    (PE is otherwise idle for ~6us waiting on weight DMAs and runs cold).
  * DMA order tuned for the ~2us per-DMA completion latency: weight blob,
    mask half 1 (diag build), chunk-0 x rows [0,18), mask half 2, rest.
  * Output bf16 (host upcasts). ~8us fixed framework postamble remains.
"""

import os
from collections import deque
from contextlib import ExitStack

import ml_dtypes
import numpy as np

import concourse.bass as bass
import concourse.mybir as mybir
import concourse.tile as tile
from concourse import bacc
from concourse.bass_utils import run_bass_kernel_spmd

AF = mybir.ActivationFunctionType
ALU = mybir.AluOpType
F32 = mybir.dt.float32
BF16 = mybir.dt.bfloat16

B, C, H, W = 8, 128, 128, 128
KK = 3
NT = KK * KK                 # 9 taps
HW = H * W
XOFF = 2                     # interior column offset in the padded layout
WP = W + 4                   # host-padded width (2 left, 2 right)
HP = H + 2                   # host-padded height (1 halo row each side)
R = 32                       # image rows per chunk
NCH = H // R                 # 4 chunks
PE_ROWS = 18                 # chunk rows [0, PE_ROWS) -> TensorE depthwise
DVE_ROWS = R - PE_ROWS       # chunk rows [PE_ROWS, R) -> Vector/Scalar E
ACT_TAPS = 3                 # tap products computed on ScalarE (chunks >= 1)
TAPS = [(dy, dx) for dy in (-1, 0, 1) for dx in (-1, 0, 1)]  # t = (dy+1)*3+(dx+1)
TAIL_LAG = 2                 # 8-row tail units kept pending (pipelining depth)
N_WARM = 16                  # PE warm-up matmuls

# bf16 weight blob column layout: w1t | alt | ca1t | ca2t | w2t | cwt
W_W1T, W_ALT, W_CA1T, W_CA2T = 0, 128, 129, 145
W_W2T, W_CWT = 145 + 128, 145 + 128 + NT * 128
W_COLS = W_CWT + 128
MASK_SPLIT = 5 * 128         # mask columns for the first diag-build half

last_results = None          # BassKernelResults of the most recent run


def _pe_blocks():
    blocks = []
    r = 0
    while r < PE_ROWS:
        rr = min(4, PE_ROWS - r)
        blocks.append((r, r + rr))
        r += rr
    return blocks


def _emit(tc, nc, d):
    ctx = d["ctx"]
    singles = ctx.enter_context(tc.tile_pool(name="singles", bufs=1))
    xpool = ctx.enter_context(tc.tile_pool(name="xpool", bufs=3))
    xbpool = ctx.enter_context(tc.tile_pool(name="xbpool", bufs=2))
    spool = ctx.enter_context(tc.tile_pool(name="spool", bufs=2))
    tpool = ctx.enter_context(tc.tile_pool(name="tpool", bufs=12))
    opool = ctx.enter_context(tc.tile_pool(name="opool", bufs=3))
    pss_pool = ctx.enter_context(tc.tile_pool(name="psum_s", bufs=4, space="PSUM"))
    pso_pool = ctx.enter_context(tc.tile_pool(name="psum_o", bufs=2, space="PSUM"))

    # -- PE warm-up: dummy matmuls on a zeroed tile, no DMA dependency --
    warm = singles.tile([128, 512], BF16, name="warm", tag="warm")
    nc.gpsimd.memset(warm, 0.0)
    wps = pso_pool.tile([128, 512], F32, name="wps", tag="pso")
    for _ in range(N_WARM):
        nc.tensor.matmul(wps, lhsT=warm[:, 0:128], rhs=warm, start=True, stop=True)

    wblob = singles.tile([128, W_COLS], BF16, name="wblob", tag="wblob")
    nc.sync.dma_start(out=wblob, in_=d["wblob"])
    mask = singles.tile([128, NT * 128], BF16, name="mask", tag="mask")
    nc.sync.dma_start(out=mask[:, 0:MASK_SPLIT], in_=d["mask"][:, 0:MASK_SPLIT])

    x3 = d["xpad"].rearrange("c (h w) -> c h w", w=WP)
    xpf_d = d["xpad"]
    NB1 = (DVE_ROWS + 2) * WP    # xb1 flat length (DVE tap rows + dy halo)

    xps, xb1s = [], []
    xp0 = xpool.tile([128, R + 2, WP], BF16, name="xp0", tag="xp")
    nc.sync.dma_start(out=xp0[:, 0:6, :], in_=x3[:, 0:6, :])
    nc.sync.dma_start(
        out=mask[:, MASK_SPLIT : NT * 128], in_=d["mask"][:, MASK_SPLIT : NT * 128]
    )
    nc.sync.dma_start(out=xp0[:, 6:18, :], in_=x3[:, 6:18, :])
    cb = singles.tile([128, 1], F32, name="cb", tag="cb")
    nc.sync.dma_start(out=cb, in_=d["cb"])
    nc.sync.dma_start(out=xp0[:, 18 : R + 2, :], in_=x3[:, 18 : R + 2, :])
    xb1_0 = xbpool.tile([128, NB1], BF16, name="xb1_0", tag="xb1")
    nc.sync.dma_start(
        out=xb1_0[:, 0 : NB1 - 2],
        in_=xpf_d[:, PE_ROWS * WP + 1 : PE_ROWS * WP + NB1 - 1],
    )
    xps.append(xp0)
    xb1s.append(xb1_0)
    _emit_prologue(tc, nc, d, singles, pss_pool, wblob, mask)
    for ci in range(1, NCH):
        y0 = ci * R
        xp = xpool.tile([128, R + 2, WP], BF16, name=f"xp{ci}", tag="xp")
        nc.sync.dma_start(out=xp, in_=x3[:, y0 : y0 + R + 2, :])
        xb1 = xbpool.tile([128, NB1], BF16, name=f"xb1{ci}", tag="xb1")
        base = (y0 + PE_ROWS) * WP
        nc.sync.dma_start(
            out=xb1[:, 0 : NB1 - 2], in_=xpf_d[:, base + 1 : base + NB1 - 1]
        )
        xps.append(xp)
        xb1s.append(xb1)

    ktab = d["ktab"]
    dg_all = d["dg_all"]
    attd = d["attd"]
    cwt = wblob[:, W_CWT : W_CWT + 128]
    out_d = d["out"]

    tails = deque()

    def flush(n):
        while len(tails) > n:
            tails.popleft()()

    for ci in range(NCH):
        y0 = ci * R
        xp = xps[ci]
        xb13 = xb1s[ci].rearrange("p (r c) -> p r c", c=WP)
        srelu = spool.tile([128, R * W], BF16, name=f"sr{ci}", tag="sr")

        # -- DVE/ACT depthwise: rows [PE_ROWS, R): 9 products, add tree --
        n_act = ACT_TAPS if ci >= 1 else 0
        prods = []
        for ti, (dy, dx) in enumerate(TAPS):
            if dx == 0:
                src = xp[:, 1 + PE_ROWS + dy : 1 + PE_ROWS + DVE_ROWS + dy,
                         XOFF : XOFF + W]
            elif dx == 1:
                src = xb13[:, 1 + dy : 1 + DVE_ROWS + dy, XOFF : XOFF + W]
            else:
                src = xb13[:, 1 + dy : 1 + DVE_ROWS + dy, 0:W]
            t = tpool.tile([128, DVE_ROWS * W], BF16, name=f"tp{ci}_{ti}", tag="tp")
            if ti < n_act:
                nc.scalar.activation(t, src, AF.Copy, scale=ktab[:, ti : ti + 1])
            else:
                nc.vector.tensor_scalar_mul(
                    out=t, in0=src, scalar1=ktab[:, ti : ti + 1]
                )
            prods.append(t)
        while len(prods) > 1:
            nxt = []
            for i in range(0, len(prods) - 1, 2):
                a, b = prods[i], prods[i + 1]
                nc.vector.tensor_tensor(out=a, in0=b, in1=a, op=ALU.add)
                nxt.append(a)
            if len(prods) % 2:
                nxt.append(prods[-1])
            prods = nxt
        nc.scalar.activation(srelu[:, PE_ROWS * W : R * W], prods[0], AF.Relu)

        # -- PE depthwise: rows [0, PE_ROWS) in <=4-row PSUM blocks --
        for rs, re in _pe_blocks():
            rows = re - rs
            pss = pss_pool.tile([128, rows * W], F32, name=f"pss{ci}_{rs}",
                                tag="pss")
            for ti, (dy, dx) in enumerate(TAPS):
                rhs = xp[:, 1 + rs + dy : 1 + re + dy, XOFF + dx : XOFF + dx + W]
                nc.tensor.matmul(
                    pss, lhsT=dg_all[:, ti * 128 : (ti + 1) * 128], rhs=rhs,
                    start=(ti == 0), stop=(ti == NT - 1),
                )
            nc.scalar.activation(srelu[:, rs * W : re * W], pss, AF.Relu)

        # -- tails: 8-row units (2 PSUM banks of pointwise+residual, 1 store) --
        last = ci == NCH - 1
        for tr in range(0, R, 8):
            tails.append(_make_tail(nc, pso_pool, opool, xp, srelu, cwt, attd,
                                    cb, out_d, ci, tr, y0))
            flush(0 if last else TAIL_LAG)
    flush(0)


def _emit_prologue(tc, nc, d, singles, pss_pool, wblob, mask):
    alt = wblob[:, W_ALT : W_ALT + 1]
    w1t = wblob[:, W_W1T : W_W1T + 128]
    ca1t = wblob[:, W_CA1T : W_CA1T + 16]
    ca2t = wblob[0:16, W_CA2T : W_CA2T + 128]
    w2t = wblob[:, W_W2T : W_W2T + NT * 128]

    def leaky(name, psum_src, parts, dt=F32):
        tmp = singles.tile([parts, 1], F32, name=f"{name}_t", tag=f"{name}_t")
        nc.scalar.activation(tmp, psum_src, AF.Copy)
        res = singles.tile([parts, 1], dt, name=name, tag=name)
        nc.vector.scalar_tensor_tensor(
            out=res, in0=tmp, scalar=0.1, in1=tmp, op0=ALU.mult, op1=ALU.max
        )
        return res

    # ---- kernel-generator MLP (all bf16 matmuls) ----
    feat_ps = pss_pool.tile([128, 1], F32, name="feat_ps", tag="pss")
    nc.tensor.matmul(feat_ps, lhsT=w1t, rhs=alt, start=True, stop=True)
    feat = leaky("feat", feat_ps, 128, dt=BF16)

    ktab_ps = pss_pool.tile([128, NT], F32, name="ktab_ps", tag="pss")
    w2r = w2t.rearrange("p (c t) -> p t c", t=NT)
    for t in range(NT):
        nc.tensor.matmul(
            ktab_ps[:, t : t + 1], lhsT=w2r[:, t, :], rhs=feat, start=True, stop=True
        )
    # SBUF copy (DVE/ACT tap scalar source) runs in parallel with the dg build.
    ktab = singles.tile([128, NT], F32, name="ktab", tag="ktab")
    nc.scalar.activation(ktab, ktab_ps, AF.Copy)

    # ---- diag weights straight from PSUM: dg[:, t*128+j] = I[p,j]*ktab[p,t]
    dg_all = singles.tile([128, NT * 128], BF16, name="dg_all", tag="dg_all")
    ktab_b = ktab_ps.unsqueeze(2).broadcast_to([128, NT, 128])
    mask3 = mask.rearrange("p (t c) -> p t c", t=NT)
    dg3 = dg_all.rearrange("p (t c) -> p t c", t=NT)
    nc.vector.tensor_tensor(
        out=dg3[:, 0:5, :], in0=mask3[:, 0:5, :], in1=ktab_b[:, 0:5, :],
        op=ALU.mult,
    )
    nc.vector.tensor_tensor(
        out=dg3[:, 5:NT, :], in0=mask3[:, 5:NT, :], in1=ktab_b[:, 5:NT, :],
        op=ALU.mult,
    )

    # ---- channel attention (bf16 matmuls) ----
    a1_ps = pss_pool.tile([16, 1], F32, name="a1_ps", tag="pss")
    nc.tensor.matmul(a1_ps, lhsT=ca1t, rhs=alt, start=True, stop=True)
    a1 = leaky("a1", a1_ps, 16, dt=BF16)
    att_ps = pss_pool.tile([128, 1], F32, name="att_ps", tag="pss")
    nc.tensor.matmul(att_ps, lhsT=ca2t, rhs=a1, start=True, stop=True)
    attv = singles.tile([128, 1], F32, name="attv", tag="attv")
    nc.scalar.activation(attv, att_ps, AF.Sigmoid)
    attd = singles.tile([128, 128], BF16, name="attd", tag="attd")
    nc.vector.tensor_tensor(
        out=attd, in0=mask[:, 0:128], in1=attv.broadcast_to([128, 128]),
        op=ALU.mult,
    )
    d["ktab"] = ktab
    d["dg_all"] = dg_all
    d["attd"] = attd


def _make_tail(nc, pso_pool, opool, xp, srelu, cwt, attd, cb, out_d, ci, tr, y0):
    """Two 4-row pointwise+residual groups into one 2-bank PSUM tile, single
    biased bf16 evac and store for chunk-relative rows [tr, tr+8)."""

    def tail():
        osb = opool.tile([128, 8 * W], BF16, name=f"ob{ci}_{tr}", tag="ob")
        pso = pso_pool.tile([128, 8 * W], F32, name=f"pso{ci}_{tr}", tag="pso")
        for h, r0 in enumerate((tr, tr + 4)):
            half = pso[:, h * 4 * W : (h + 1) * 4 * W]
            nc.tensor.matmul(half, lhsT=cwt, rhs=srelu[:, r0 * W : (r0 + 4) * W],
                             start=True, stop=False)
            nc.tensor.matmul(
                half, lhsT=attd, rhs=xp[:, 1 + r0 : 1 + r0 + 4, XOFF : XOFF + W],
                start=False, stop=True,
            )
        nc.scalar.activation(osb, pso, AF.Identity, bias=cb)
        nc.sync.dma_start(
            out=out_d[:, (y0 + tr) * W : (y0 + tr + 8) * W], in_=osb
        )

    return tail


def build_module():
    nc = bacc.Bacc(
        "TRN2",
        target_bir_lowering=False,
        debug=False,
        enable_asserts=False,
        num_devices=B,
    )
    d = {
        "xpad": nc.dram_tensor("xpad", [C, HP * WP], BF16, kind="ExternalInput").ap(),
        "wblob": nc.dram_tensor("wblob", [128, W_COLS], BF16, kind="ExternalInput").ap(),
        "cb": nc.dram_tensor("cb", [C, 1], F32, kind="ExternalInput").ap(),
        "mask": nc.dram_tensor("mask", [128, NT * 128], BF16, kind="ExternalInput").ap(),
        "out": nc.dram_tensor("out", [C, HW], BF16, kind="ExternalOutput").ap(),
    }
    with tile.TileContext(nc) as tc:
        with ExitStack() as ctx:
            d["ctx"] = ctx
            _emit(tc, nc, d)
    nc.finalize()
    return nc


_module_cache = None


def _get_module():
    global _module_cache
    if _module_cache is None:
        _module_cache = build_module()
    return _module_cache


def make_in_maps(x, altitude, W1, W2, conv_w, conv_b, ca_w1, ca_w2):
    f = np.float32
    bf = ml_dtypes.bfloat16
    x = np.asarray(x, dtype=f)
    altitude = np.asarray(altitude, dtype=f)
    xpad = np.zeros((B, C, HP, WP), dtype=f)
    xpad[:, :, 1 : H + 1, XOFF : XOFF + W] = x
    xq = np.ascontiguousarray(xpad.astype(bf).reshape(B, C, HP * WP))

    wblob_shared = np.zeros((128, W_COLS), dtype=bf)
    wblob_shared[:, W_W1T : W_W1T + 128] = np.asarray(W1, dtype=f).T.astype(bf)
    wblob_shared[:, W_CA1T : W_CA1T + 16] = np.asarray(ca_w1, dtype=f).T.astype(bf)
    wblob_shared[0:16, W_CA2T : W_CA2T + 128] = np.asarray(
        ca_w2, dtype=f
    ).T.astype(bf)
    wblob_shared[:, W_W2T : W_W2T + NT * 128] = np.asarray(
        W2, dtype=f
    ).T.astype(bf)
    wblob_shared[:, W_CWT : W_CWT + 128] = np.asarray(conv_w, dtype=f).T.astype(bf)

    cb_arr = np.ascontiguousarray(np.asarray(conv_b, dtype=f).reshape(C, 1))
    mask_arr = np.ascontiguousarray(
        np.tile(np.eye(128, dtype=f), (1, NT)).astype(bf)
    )

    maps = []
    for bb in range(B):
        wblob = wblob_shared.copy()
        wblob[:, W_ALT] = altitude[bb].astype(bf)
        maps.append({"xpad": xq[bb], "wblob": np.ascontiguousarray(wblob),
                     "cb": cb_arr, "mask": mask_arr})
    return maps


def kernel(x, altitude, W1, W2, conv_w, conv_b, ca_w1, ca_w2):
    global last_results
    in_maps = make_in_maps(x, altitude, W1, W2, conv_w, conv_b, ca_w1, ca_w2)
    nc = _get_module()
    trace = os.environ.get("KERNEL_TRACE", "0") == "1"
    last_results = run_bass_kernel_spmd(
        nc, in_maps, core_ids=list(range(B)), trace=trace
    )
    out = np.stack(
        [
            last_results.results[bb]["out"].astype(np.float32).reshape(C, H, W)
            for bb in range(B)
        ]
    )
    return out


# revision 9
# speedup vs baseline: 1.1229x; 1.0583x over previous
"""Trainium2 Bass kernel for nn_DA_conv: per-sample generated depthwise 3x3 conv
-> relu -> 1x1 pointwise conv (+bias) -> + x * channel_attention(altitude).

Data-parallel over batch: 8 samples -> 8 NeuronCores, weights replicated.

v5 design (trace-driven, from the 81.8us v4):
  * Depthwise three-way split per 32-row chunk: TensorE rows [0,19) (diag bf16
    matmuls), VectorE rows [19,32) (tensor_scalar_mul 4x + depth-4
    tensor_tensor add tree 2x), ScalarE computes 2 of the 9 tap products
    (activation Copy with per-partition scale) on chunks 1-3.
    Measured: TS = 208+0.266N, TT = 153+0.518N, MM N=512 gap 218ns.
  * Residual x*att rides the pointwise PSUM group as a diag(att) matmul.
  * ~16 dummy warm-up matmuls on a zeroed tile right after the preamble
    barrier: keeps the PE HAM clock at 2.4GHz before the first real taps
    (PE is otherwise idle for ~6us waiting on weight DMAs and runs cold).
  * DMA order tuned for the ~2us per-DMA completion latency: weight blob,
    mask half 1 (diag build), chunk-0 x rows [0,18), mask half 2, rest.
  * Output bf16 (host upcasts). ~8us fixed framework postamble remains.
"""

import os
from collections import deque
from contextlib import ExitStack

import ml_dtypes
import numpy as np

import concourse.bass as bass
import concourse.mybir as mybir
import concourse.tile as tile
from concourse import bacc
from concourse.bass_utils import run_bass_kernel_spmd

AF = mybir.ActivationFunctionType
ALU = mybir.AluOpType
F32 = mybir.dt.float32
BF16 = mybir.dt.bfloat16

B, C, H, W = 8, 128, 128, 128
KK = 3
NT = KK * KK                 # 9 taps
HW = H * W
XOFF = 2                     # interior column offset in the padded layout
WP = W + 4                   # host-padded width (2 left, 2 right)
HP = H + 2                   # host-padded height (1 halo row each side)
R = 32                       # image rows per chunk
NCH = H // R                 # 4 chunks
PE_ROWS = 20                 # chunk rows [0, PE_ROWS) -> TensorE depthwise
DVE_ROWS = R - PE_ROWS       # chunk rows [PE_ROWS, R) -> Vector/Scalar E
ACT_TAPS = 0                 # tap products computed on ScalarE (chunks >= 1)
TAPS = [(dy, dx) for dy in (-1, 0, 1) for dx in (-1, 0, 1)]  # t = (dy+1)*3+(dx+1)
TAIL_LAG = 2                 # 8-row tail units kept pending (pipelining depth)
N_WARM = 16                  # PE warm-up matmuls

# bf16 weight blob column layout: w1t | alt | ca1t | ca2t | w2t | cwt
W_W1T, W_ALT, W_CA1T, W_CA2T = 0, 128, 129, 145
W_W2T, W_CWT = 145 + 128, 145 + 128 + NT * 128
W_COLS = W_CWT + 128
MASK_SPLIT = 5 * 128         # mask columns for the first diag-build half

last_results = None          # BassKernelResults of the most recent run


def _pe_blocks():
    blocks = []
    r = 0
    while r < PE_ROWS:
        rr = min(4, PE_ROWS - r)
        blocks.append((r, r + rr))
        r += rr
    return blocks


def _emit(tc, nc, d):
    ctx = d["ctx"]
    singles = ctx.enter_context(tc.tile_pool(name="singles", bufs=1))
    xpool = ctx.enter_context(tc.tile_pool(name="xpool", bufs=3))
    xbpool = ctx.enter_context(tc.tile_pool(name="xbpool", bufs=2))
    spool = ctx.enter_context(tc.tile_pool(name="spool", bufs=2))
    tpool = ctx.enter_context(tc.tile_pool(name="tpool", bufs=12))
    opool = ctx.enter_context(tc.tile_pool(name="opool", bufs=3))
    pss_pool = ctx.enter_context(tc.tile_pool(name="psum_s", bufs=4, space="PSUM"))
    pso_pool = ctx.enter_context(tc.tile_pool(name="psum_o", bufs=2, space="PSUM"))

    # -- PE warm-up: dummy matmuls on a zeroed tile, no DMA dependency --
    warm = singles.tile([128, 512], BF16, name="warm", tag="warm")
    nc.gpsimd.memset(warm, 0.0)
    wps = pso_pool.tile([128, 512], F32, name="wps", tag="pso")
    for _ in range(N_WARM):
        nc.tensor.matmul(wps, lhsT=warm[:, 0:128], rhs=warm, start=True, stop=True)

    wblob = singles.tile([128, W_COLS], BF16, name="wblob", tag="wblob")
    nc.sync.dma_start(out=wblob, in_=d["wblob"])
    mask = singles.tile([128, NT * 128], BF16, name="mask", tag="mask")
    nc.sync.dma_start(out=mask[:, 0:MASK_SPLIT], in_=d["mask"][:, 0:MASK_SPLIT])

    x3 = d["xpad"].rearrange("c (h w) -> c h w", w=WP)
    xpf_d = d["xpad"]
    NB1 = (DVE_ROWS + 2) * WP    # xb1 flat length (DVE tap rows + dy halo)

    xps, xb1s = [], []
    xp0 = xpool.tile([128, R + 2, WP], BF16, name="xp0", tag="xp")
    nc.sync.dma_start(out=xp0[:, 0:6, :], in_=x3[:, 0:6, :])
    nc.sync.dma_start(
        out=mask[:, MASK_SPLIT : NT * 128], in_=d["mask"][:, MASK_SPLIT : NT * 128]
    )
    nc.sync.dma_start(out=xp0[:, 6:18, :], in_=x3[:, 6:18, :])
    cb = singles.tile([128, 1], F32, name="cb", tag="cb")
    nc.sync.dma_start(out=cb, in_=d["cb"])
    nc.sync.dma_start(out=xp0[:, 18 : R + 2, :], in_=x3[:, 18 : R + 2, :])
    xb1_0 = xbpool.tile([128, NB1], BF16, name="xb1_0", tag="xb1")
    nc.sync.dma_start(
        out=xb1_0[:, 0 : NB1 - 2],
        in_=xpf_d[:, PE_ROWS * WP + 1 : PE_ROWS * WP + NB1 - 1],
    )
    xps.append(xp0)
    xb1s.append(xb1_0)
    _emit_prologue(tc, nc, d, singles, pss_pool, wblob, mask)
    for ci in range(1, NCH):
        y0 = ci * R
        xp = xpool.tile([128, R + 2, WP], BF16, name=f"xp{ci}", tag="xp")
        nc.sync.dma_start(out=xp, in_=x3[:, y0 : y0 + R + 2, :])
        xb1 = xbpool.tile([128, NB1], BF16, name=f"xb1{ci}", tag="xb1")
        base = (y0 + PE_ROWS) * WP
        nc.sync.dma_start(
            out=xb1[:, 0 : NB1 - 2], in_=xpf_d[:, base + 1 : base + NB1 - 1]
        )
        xps.append(xp)
        xb1s.append(xb1)

    ktab = d["ktab"]
    dg_all = d["dg_all"]
    attd = d["attd"]
    cwt = wblob[:, W_CWT : W_CWT + 128]
    out_d = d["out"]

    tails = deque()

    def flush(n):
        while len(tails) > n:
            tails.popleft()()

    for ci in range(NCH):
        y0 = ci * R
        xp = xps[ci]
        xb13 = xb1s[ci].rearrange("p (r c) -> p r c", c=WP)
        srelu = spool.tile([128, R * W], BF16, name=f"sr{ci}", tag="sr")

        # -- DVE/ACT depthwise: rows [PE_ROWS, R): 9 products, add tree --
        n_act = ACT_TAPS if ci >= 1 else 0
        prods = []
        for ti, (dy, dx) in enumerate(TAPS):
            if dx == 0:
                src = xp[:, 1 + PE_ROWS + dy : 1 + PE_ROWS + DVE_ROWS + dy,
                         XOFF : XOFF + W]
            elif dx == 1:
                src = xb13[:, 1 + dy : 1 + DVE_ROWS + dy, XOFF : XOFF + W]
            else:
                src = xb13[:, 1 + dy : 1 + DVE_ROWS + dy, 0:W]
            t = tpool.tile([128, DVE_ROWS * W], BF16, name=f"tp{ci}_{ti}", tag="tp")
            if ti < n_act:
                nc.scalar.activation(t, src, AF.Copy, scale=ktab[:, ti : ti + 1])
            else:
                nc.vector.tensor_scalar_mul(
                    out=t, in0=src, scalar1=ktab[:, ti : ti + 1]
                )
            prods.append(t)
        while len(prods) > 1:
            nxt = []
            for i in range(0, len(prods) - 1, 2):
                a, b = prods[i], prods[i + 1]
                nc.vector.tensor_tensor(out=a, in0=b, in1=a, op=ALU.add)
                nxt.append(a)
            if len(prods) % 2:
                nxt.append(prods[-1])
            prods = nxt
        nc.scalar.activation(srelu[:, PE_ROWS * W : R * W], prods[0], AF.Relu)

        # -- PE depthwise: rows [0, PE_ROWS) in <=4-row PSUM blocks --
        for rs, re in _pe_blocks():
            rows = re - rs
            pss = pss_pool.tile([128, rows * W], F32, name=f"pss{ci}_{rs}",
                                tag="pss")
            for ti, (dy, dx) in enumerate(TAPS):
                rhs = xp[:, 1 + rs + dy : 1 + re + dy, XOFF + dx : XOFF + dx + W]
                nc.tensor.matmul(
                    pss, lhsT=dg_all[:, ti * 128 : (ti + 1) * 128], rhs=rhs,
                    start=(ti == 0), stop=(ti == NT - 1),
                )
            nc.scalar.activation(srelu[:, rs * W : re * W], pss, AF.Relu)

        # -- tails: 8-row units (2 PSUM banks of pointwise+residual, 1 store) --
        last = ci == NCH - 1
        for tr in range(0, R, 8):
            tails.append(_make_tail(nc, pso_pool, opool, xp, srelu, cwt, attd,
                                    cb, out_d, ci, tr, y0))
            flush(0 if last else TAIL_LAG)
    flush(0)


def _emit_prologue(tc, nc, d, singles, pss_pool, wblob, mask):
    alt = wblob[:, W_ALT : W_ALT + 1]
    w1t = wblob[:, W_W1T : W_W1T + 128]
    ca1t = wblob[:, W_CA1T : W_CA1T + 16]
    ca2t = wblob[0:16, W_CA2T : W_CA2T + 128]
    w2t = wblob[:, W_W2T : W_W2T + NT * 128]

    def leaky(name, psum_src, parts, dt=F32):
        tmp = singles.tile([parts, 1], F32, name=f"{name}_t", tag=f"{name}_t")
        nc.scalar.activation(tmp, psum_src, AF.Copy)
        res = singles.tile([parts, 1], dt, name=name, tag=name)
        nc.vector.scalar_tensor_tensor(
            out=res, in0=tmp, scalar=0.1, in1=tmp, op0=ALU.mult, op1=ALU.max
        )
        return res

    # ---- kernel-generator MLP (all bf16 matmuls) ----
    feat_ps = pss_pool.tile([128, 1], F32, name="feat_ps", tag="pss")
    nc.tensor.matmul(feat_ps, lhsT=w1t, rhs=alt, start=True, stop=True)
    feat = leaky("feat", feat_ps, 128, dt=BF16)

    ktab_ps = pss_pool.tile([128, NT], F32, name="ktab_ps", tag="pss")
    w2r = w2t.rearrange("p (c t) -> p t c", t=NT)
    for t in range(NT):
        nc.tensor.matmul(
            ktab_ps[:, t : t + 1], lhsT=w2r[:, t, :], rhs=feat, start=True, stop=True
        )
    # SBUF copy (DVE/ACT tap scalar source) runs in parallel with the dg build.
    ktab = singles.tile([128, NT], F32, name="ktab", tag="ktab")
    nc.scalar.activation(ktab, ktab_ps, AF.Copy)

    # ---- diag weights straight from PSUM: dg[:, t*128+j] = I[p,j]*ktab[p,t]
    dg_all = singles.tile([128, NT * 128], BF16, name="dg_all", tag="dg_all")
    ktab_b = ktab_ps.unsqueeze(2).broadcast_to([128, NT, 128])
    mask3 = mask.rearrange("p (t c) -> p t c", t=NT)
    dg3 = dg_all.rearrange("p (t c) -> p t c", t=NT)
    nc.vector.tensor_tensor(
        out=dg3[:, 0:5, :], in0=mask3[:, 0:5, :], in1=ktab_b[:, 0:5, :],
        op=ALU.mult,
    )
    nc.vector.tensor_tensor(
        out=dg3[:, 5:NT, :], in0=mask3[:, 5:NT, :], in1=ktab_b[:, 5:NT, :],
        op=ALU.mult,
    )

    # ---- channel attention (bf16 matmuls) ----
    a1_ps = pss_pool.tile([16, 1], F32, name="a1_ps", tag="pss")
    nc.tensor.matmul(a1_ps, lhsT=ca1t, rhs=alt, start=True, stop=True)
    a1 = leaky("a1", a1_ps, 16, dt=BF16)
    att_ps = pss_pool.tile([128, 1], F32, name="att_ps", tag="pss")
    nc.tensor.matmul(att_ps, lhsT=ca2t, rhs=a1, start=True, stop=True)
    attv = singles.tile([128, 1], F32, name="attv", tag="attv")
    nc.scalar.activation(attv, att_ps, AF.Sigmoid)
    attd = singles.tile([128, 128], BF16, name="attd", tag="attd")
    nc.vector.tensor_tensor(
        out=attd, in0=mask[:, 0:128], in1=attv.broadcast_to([128, 128]),
        op=ALU.mult,
    )
    d["ktab"] = ktab
    d["dg_all"] = dg_all
    d["attd"] = attd


def _make_tail(nc, pso_pool, opool, xp, srelu, cwt, attd, cb, out_d, ci, tr, y0):
    """Two 4-row pointwise+residual groups into one 2-bank PSUM tile, single
    biased bf16 evac and store for chunk-relative rows [tr, tr+8)."""

    def tail():
        osb = opool.tile([128, 8 * W], BF16, name=f"ob{ci}_{tr}", tag="ob")
        pso = pso_pool.tile([128, 8 * W], F32, name=f"pso{ci}_{tr}", tag="pso")
        for h, r0 in enumerate((tr, tr + 4)):
            half = pso[:, h * 4 * W : (h + 1) * 4 * W]
            nc.tensor.matmul(half, lhsT=cwt, rhs=srelu[:, r0 * W : (r0 + 4) * W],
                             start=True, stop=False)
            nc.tensor.matmul(
                half, lhsT=attd, rhs=xp[:, 1 + r0 : 1 + r0 + 4, XOFF : XOFF + W],
                start=False, stop=True,
            )
        nc.scalar.activation(osb, pso, AF.Identity, bias=cb)
        nc.gpsimd.dma_start(
            out=out_d[:, (y0 + tr) * W : (y0 + tr + 8) * W], in_=osb
        )

    return tail


def build_module():
    nc = bacc.Bacc(
        "TRN2",
        target_bir_lowering=False,
        debug=False,
        enable_asserts=False,
        num_devices=B,
    )
    d = {
        "xpad": nc.dram_tensor("xpad", [C, HP * WP], BF16, kind="ExternalInput").ap(),
        "wblob": nc.dram_tensor("wblob", [128, W_COLS], BF16, kind="ExternalInput").ap(),
        "cb": nc.dram_tensor("cb", [C, 1], F32, kind="ExternalInput").ap(),
        "mask": nc.dram_tensor("mask", [128, NT * 128], BF16, kind="ExternalInput").ap(),
        "out": nc.dram_tensor("out", [C, HW], BF16, kind="ExternalOutput").ap(),
    }
    with tile.TileContext(nc) as tc:
        with ExitStack() as ctx:
            d["ctx"] = ctx
            _emit(tc, nc, d)
    nc.finalize()
    return nc


_module_cache = None


def _get_module():
    global _module_cache
    if _module_cache is None:
        _module_cache = build_module()
    return _module_cache


def make_in_maps(x, altitude, W1, W2, conv_w, conv_b, ca_w1, ca_w2):
    f = np.float32
    bf = ml_dtypes.bfloat16
    x = np.asarray(x, dtype=f)
    altitude = np.asarray(altitude, dtype=f)
    xpad = np.zeros((B, C, HP, WP), dtype=f)
    xpad[:, :, 1 : H + 1, XOFF : XOFF + W] = x
    xq = np.ascontiguousarray(xpad.astype(bf).reshape(B, C, HP * WP))

    wblob_shared = np.zeros((128, W_COLS), dtype=bf)
    wblob_shared[:, W_W1T : W_W1T + 128] = np.asarray(W1, dtype=f).T.astype(bf)
    wblob_shared[:, W_CA1T : W_CA1T + 16] = np.asarray(ca_w1, dtype=f).T.astype(bf)
    wblob_shared[0:16, W_CA2T : W_CA2T + 128] = np.asarray(
        ca_w2, dtype=f
    ).T.astype(bf)
    wblob_shared[:, W_W2T : W_W2T + NT * 128] = np.asarray(
        W2, dtype=f
    ).T.astype(bf)
    wblob_shared[:, W_CWT : W_CWT + 128] = np.asarray(conv_w, dtype=f).T.astype(bf)

    cb_arr = np.ascontiguousarray(np.asarray(conv_b, dtype=f).reshape(C, 1))
    mask_arr = np.ascontiguousarray(
        np.tile(np.eye(128, dtype=f), (1, NT)).astype(bf)
    )

    maps = []
    for bb in range(B):
        wblob = wblob_shared.copy()
        wblob[:, W_ALT] = altitude[bb].astype(bf)
        maps.append({"xpad": xq[bb], "wblob": np.ascontiguousarray(wblob),
                     "cb": cb_arr, "mask": mask_arr})
    return maps


def kernel(x, altitude, W1, W2, conv_w, conv_b, ca_w1, ca_w2):
    global last_results
    in_maps = make_in_maps(x, altitude, W1, W2, conv_w, conv_b, ca_w1, ca_w2)
    nc = _get_module()
    trace = os.environ.get("KERNEL_TRACE", "0") == "1"
    last_results = run_bass_kernel_spmd(
        nc, in_maps, core_ids=list(range(B)), trace=trace
    )
    out = np.stack(
        [
            last_results.results[bb]["out"].astype(np.float32).reshape(C, H, W)
            for bb in range(B)
        ]
    )
    return out


# revision 10
# speedup vs baseline: 1.1320x; 1.0081x over previous
"""Trainium2 Bass kernel for nn_DA_conv: per-sample generated depthwise 3x3 conv
-> relu -> 1x1 pointwise conv (+bias) -> + x * channel_attention(altitude).

Data-parallel over batch: 8 samples -> 8 NeuronCores, weights replicated.

v5 design (trace-driven, from the 81.8us v4):
  * Depthwise three-way split per 32-row chunk: TensorE rows [0,19) (diag bf16
    matmuls), VectorE rows [19,32) (tensor_scalar_mul 4x + depth-4
    tensor_tensor add tree 2x), ScalarE computes 2 of the 9 tap products
    (activation Copy with per-partition scale) on chunks 1-3.
    Measured: TS = 208+0.266N, TT = 153+0.518N, MM N=512 gap 218ns.
  * Residual x*att rides the pointwise PSUM group as a diag(att) matmul.
  * ~16 dummy warm-up matmuls on a zeroed tile right after the preamble
    barrier: keeps the PE HAM clock at 2.4GHz before the first real taps
    (PE is otherwise idle for ~6us waiting on weight DMAs and runs cold).
  * DMA order tuned for the ~2us per-DMA completion latency: weight blob,
    mask half 1 (diag build), chunk-0 x rows [0,18), mask half 2, rest.
  * Output bf16 (host upcasts). ~8us fixed framework postamble remains.
"""

import os
from collections import deque
from contextlib import ExitStack

import ml_dtypes
import numpy as np

import concourse.bass as bass
import concourse.mybir as mybir
import concourse.tile as tile
from concourse import bacc
from concourse.bass_utils import run_bass_kernel_spmd

AF = mybir.ActivationFunctionType
ALU = mybir.AluOpType
F32 = mybir.dt.float32
BF16 = mybir.dt.bfloat16

B, C, H, W = 8, 128, 128, 128
KK = 3
NT = KK * KK                 # 9 taps
HW = H * W
XOFF = 2                     # interior column offset in the padded layout
WP = W + 4                   # host-padded width (2 left, 2 right)
HP = H + 2                   # host-padded height (1 halo row each side)
R = 32                       # image rows per chunk
NCH = H // R                 # 4 chunks
PE_ROWS = 20                 # chunk rows [0, PE_ROWS) -> TensorE depthwise
DVE_ROWS = R - PE_ROWS       # chunk rows [PE_ROWS, R) -> Vector/Scalar E
ACT_TAPS = 0                 # tap products computed on ScalarE (chunks >= 1)
TAPS = [(dy, dx) for dy in (-1, 0, 1) for dx in (-1, 0, 1)]  # t = (dy+1)*3+(dx+1)
TAIL_LAG = 2                 # 8-row tail units kept pending (pipelining depth)
N_WARM = 16                  # PE warm-up matmuls

# bf16 weight blob a: w1t | alt | ca1t | ca2t   (small, lands first)
W_W1T, W_ALT, W_CA1T, W_CA2T = 0, 128, 129, 145
WA_COLS = 145 + 128
# bf16 weight blob b: w2t | cwt
W_W2T, W_CWT = 0, NT * 128
WB_COLS = NT * 128 + 128
MASK_SPLIT = 5 * 128         # mask columns for the first diag-build half

last_results = None          # BassKernelResults of the most recent run


def _pe_blocks():
    blocks = []
    r = 0
    while r < PE_ROWS:
        rr = min(4, PE_ROWS - r)
        blocks.append((r, r + rr))
        r += rr
    return blocks


def _emit(tc, nc, d):
    ctx = d["ctx"]
    singles = ctx.enter_context(tc.tile_pool(name="singles", bufs=1))
    xpool = ctx.enter_context(tc.tile_pool(name="xpool", bufs=3))
    xbpool = ctx.enter_context(tc.tile_pool(name="xbpool", bufs=2))
    spool = ctx.enter_context(tc.tile_pool(name="spool", bufs=2))
    tpool = ctx.enter_context(tc.tile_pool(name="tpool", bufs=12))
    opool = ctx.enter_context(tc.tile_pool(name="opool", bufs=3))
    pss_pool = ctx.enter_context(tc.tile_pool(name="psum_s", bufs=4, space="PSUM"))
    pso_pool = ctx.enter_context(tc.tile_pool(name="psum_o", bufs=2, space="PSUM"))

    # -- PE warm-up: dummy matmuls on a zeroed tile, no DMA dependency --
    warm = singles.tile([128, 512], BF16, name="warm", tag="warm")
    nc.gpsimd.memset(warm, 0.0)
    wps = pso_pool.tile([128, 512], F32, name="wps", tag="pso")
    for _ in range(N_WARM):
        nc.tensor.matmul(wps, lhsT=warm[:, 0:128], rhs=warm, start=True, stop=True)

    wblob_a = singles.tile([128, WA_COLS], BF16, name="wblob_a", tag="wblob_a")
    nc.sync.dma_start(out=wblob_a, in_=d["wblob_a"])
    wblob_b = singles.tile([128, WB_COLS], BF16, name="wblob_b", tag="wblob_b")
    nc.sync.dma_start(out=wblob_b, in_=d["wblob_b"])
    mask = singles.tile([128, NT * 128], BF16, name="mask", tag="mask")
    nc.sync.dma_start(out=mask[:, 0:MASK_SPLIT], in_=d["mask"][:, 0:MASK_SPLIT])

    x3 = d["xpad"].rearrange("c (h w) -> c h w", w=WP)
    xpf_d = d["xpad"]
    NB1 = (DVE_ROWS + 2) * WP    # xb1 flat length (DVE tap rows + dy halo)

    xps, xb1s = [], []
    xp0 = xpool.tile([128, R + 2, WP], BF16, name="xp0", tag="xp")
    nc.sync.dma_start(out=xp0[:, 0:6, :], in_=x3[:, 0:6, :])
    nc.sync.dma_start(
        out=mask[:, MASK_SPLIT : NT * 128], in_=d["mask"][:, MASK_SPLIT : NT * 128]
    )
    nc.sync.dma_start(out=xp0[:, 6:18, :], in_=x3[:, 6:18, :])
    cb = singles.tile([128, 1], F32, name="cb", tag="cb")
    nc.sync.dma_start(out=cb, in_=d["cb"])
    nc.sync.dma_start(out=xp0[:, 18 : R + 2, :], in_=x3[:, 18 : R + 2, :])
    xb1_0 = xbpool.tile([128, NB1], BF16, name="xb1_0", tag="xb1")
    nc.sync.dma_start(
        out=xb1_0[:, 0 : NB1 - 2],
        in_=xpf_d[:, PE_ROWS * WP + 1 : PE_ROWS * WP + NB1 - 1],
    )
    xps.append(xp0)
    xb1s.append(xb1_0)
    _emit_prologue(tc, nc, d, singles, pss_pool, wblob_a, wblob_b, mask)
    for ci in range(1, NCH):
        y0 = ci * R
        xp = xpool.tile([128, R + 2, WP], BF16, name=f"xp{ci}", tag="xp")
        nc.sync.dma_start(out=xp, in_=x3[:, y0 : y0 + R + 2, :])
        xb1 = xbpool.tile([128, NB1], BF16, name=f"xb1{ci}", tag="xb1")
        base = (y0 + PE_ROWS) * WP
        nc.sync.dma_start(
            out=xb1[:, 0 : NB1 - 2], in_=xpf_d[:, base + 1 : base + NB1 - 1]
        )
        xps.append(xp)
        xb1s.append(xb1)

    ktab = d["ktab"]
    dg_all = d["dg_all"]
    attd = d["attd"]
    cwt = wblob_b[:, W_CWT : W_CWT + 128]
    out_d = d["out"]

    tails = deque()

    def flush(n):
        while len(tails) > n:
            tails.popleft()()

    for ci in range(NCH):
        y0 = ci * R
        xp = xps[ci]
        xb13 = xb1s[ci].rearrange("p (r c) -> p r c", c=WP)
        srelu = spool.tile([128, R * W], BF16, name=f"sr{ci}", tag="sr")

        # -- DVE/ACT depthwise: rows [PE_ROWS, R): 9 products, add tree --
        n_act = ACT_TAPS if ci >= 1 else 0
        prods = []
        for ti, (dy, dx) in enumerate(TAPS):
            if dx == 0:
                src = xp[:, 1 + PE_ROWS + dy : 1 + PE_ROWS + DVE_ROWS + dy,
                         XOFF : XOFF + W]
            elif dx == 1:
                src = xb13[:, 1 + dy : 1 + DVE_ROWS + dy, XOFF : XOFF + W]
            else:
                src = xb13[:, 1 + dy : 1 + DVE_ROWS + dy, 0:W]
            t = tpool.tile([128, DVE_ROWS * W], BF16, name=f"tp{ci}_{ti}", tag="tp")
            if ti < n_act:
                nc.scalar.activation(t, src, AF.Copy, scale=ktab[:, ti : ti + 1])
            else:
                nc.vector.tensor_scalar_mul(
                    out=t, in0=src, scalar1=ktab[:, ti : ti + 1]
                )
            prods.append(t)
        while len(prods) > 1:
            nxt = []
            for i in range(0, len(prods) - 1, 2):
                a, b = prods[i], prods[i + 1]
                nc.vector.tensor_tensor(out=a, in0=b, in1=a, op=ALU.add)
                nxt.append(a)
            if len(prods) % 2:
                nxt.append(prods[-1])
            prods = nxt
        nc.scalar.activation(srelu[:, PE_ROWS * W : R * W], prods[0], AF.Relu)

        # -- PE depthwise: rows [0, PE_ROWS) in <=4-row PSUM blocks --
        for rs, re in _pe_blocks():
            rows = re - rs
            pss = pss_pool.tile([128, rows * W], F32, name=f"pss{ci}_{rs}",
                                tag="pss")
            for ti, (dy, dx) in enumerate(TAPS):
                rhs = xp[:, 1 + rs + dy : 1 + re + dy, XOFF + dx : XOFF + dx + W]
                nc.tensor.matmul(
                    pss, lhsT=dg_all[:, ti * 128 : (ti + 1) * 128], rhs=rhs,
                    start=(ti == 0), stop=(ti == NT - 1),
                )
            nc.scalar.activation(srelu[:, rs * W : re * W], pss, AF.Relu)

        # -- tails: 8-row units (2 PSUM banks of pointwise+residual, 1 store) --
        last = ci == NCH - 1
        for tr in range(0, R, 8):
            tails.append(_make_tail(nc, pso_pool, opool, xp, srelu, cwt, attd,
                                    cb, out_d, ci, tr, y0))
            flush(0 if last else TAIL_LAG)
    flush(0)


def _emit_prologue(tc, nc, d, singles, pss_pool, wblob_a, wblob_b, mask):
    alt = wblob_a[:, W_ALT : W_ALT + 1]
    w1t = wblob_a[:, W_W1T : W_W1T + 128]
    ca1t = wblob_a[:, W_CA1T : W_CA1T + 16]
    ca2t = wblob_a[0:16, W_CA2T : W_CA2T + 128]
    w2t = wblob_b[:, W_W2T : W_W2T + NT * 128]

    def leaky(name, psum_src, parts, dt=F32):
        tmp = singles.tile([parts, 1], F32, name=f"{name}_t", tag=f"{name}_t")
        nc.scalar.activation(tmp, psum_src, AF.Copy)
        res = singles.tile([parts, 1], dt, name=name, tag=name)
        nc.vector.scalar_tensor_tensor(
            out=res, in0=tmp, scalar=0.1, in1=tmp, op0=ALU.mult, op1=ALU.max
        )
        return res

    # ---- kernel-generator MLP (all bf16 matmuls) ----
    feat_ps = pss_pool.tile([128, 1], F32, name="feat_ps", tag="pss")
    nc.tensor.matmul(feat_ps, lhsT=w1t, rhs=alt, start=True, stop=True)
    feat = leaky("feat", feat_ps, 128, dt=BF16)

    ktab_ps = pss_pool.tile([128, NT], F32, name="ktab_ps", tag="pss")
    w2r = w2t.rearrange("p (c t) -> p t c", t=NT)
    for t in range(NT):
        nc.tensor.matmul(
            ktab_ps[:, t : t + 1], lhsT=w2r[:, t, :], rhs=feat, start=True, stop=True
        )
    # SBUF copy (DVE/ACT tap scalar source) runs in parallel with the dg build.
    ktab = singles.tile([128, NT], F32, name="ktab", tag="ktab")
    nc.scalar.activation(ktab, ktab_ps, AF.Copy)

    # ---- diag weights straight from PSUM: dg[:, t*128+j] = I[p,j]*ktab[p,t]
    dg_all = singles.tile([128, NT * 128], BF16, name="dg_all", tag="dg_all")
    ktab_b = ktab_ps.unsqueeze(2).broadcast_to([128, NT, 128])
    mask3 = mask.rearrange("p (t c) -> p t c", t=NT)
    dg3 = dg_all.rearrange("p (t c) -> p t c", t=NT)
    nc.vector.tensor_tensor(
        out=dg3[:, 0:5, :], in0=mask3[:, 0:5, :], in1=ktab_b[:, 0:5, :],
        op=ALU.mult,
    )
    nc.vector.tensor_tensor(
        out=dg3[:, 5:NT, :], in0=mask3[:, 5:NT, :], in1=ktab_b[:, 5:NT, :],
        op=ALU.mult,
    )

    # ---- channel attention (bf16 matmuls) ----
    a1_ps = pss_pool.tile([16, 1], F32, name="a1_ps", tag="pss")
    nc.tensor.matmul(a1_ps, lhsT=ca1t, rhs=alt, start=True, stop=True)
    a1 = leaky("a1", a1_ps, 16, dt=BF16)
    att_ps = pss_pool.tile([128, 1], F32, name="att_ps", tag="pss")
    nc.tensor.matmul(att_ps, lhsT=ca2t, rhs=a1, start=True, stop=True)
    attv = singles.tile([128, 1], F32, name="attv", tag="attv")
    nc.scalar.activation(attv, att_ps, AF.Sigmoid)
    attd = singles.tile([128, 128], BF16, name="attd", tag="attd")
    nc.vector.tensor_tensor(
        out=attd, in0=mask[:, 0:128], in1=attv.broadcast_to([128, 128]),
        op=ALU.mult,
    )
    d["ktab"] = ktab
    d["dg_all"] = dg_all
    d["attd"] = attd


def _make_tail(nc, pso_pool, opool, xp, srelu, cwt, attd, cb, out_d, ci, tr, y0):
    """Two 4-row pointwise+residual groups into one 2-bank PSUM tile, single
    biased bf16 evac and store for chunk-relative rows [tr, tr+8)."""

    def tail():
        osb = opool.tile([128, 8 * W], BF16, name=f"ob{ci}_{tr}", tag="ob")
        pso = pso_pool.tile([128, 8 * W], F32, name=f"pso{ci}_{tr}", tag="pso")
        for h, r0 in enumerate((tr, tr + 4)):
            half = pso[:, h * 4 * W : (h + 1) * 4 * W]
            nc.tensor.matmul(half, lhsT=cwt, rhs=srelu[:, r0 * W : (r0 + 4) * W],
                             start=True, stop=False)
            nc.tensor.matmul(
                half, lhsT=attd, rhs=xp[:, 1 + r0 : 1 + r0 + 4, XOFF : XOFF + W],
                start=False, stop=True,
            )
        nc.scalar.activation(osb, pso, AF.Identity, bias=cb)
        nc.gpsimd.dma_start(
            out=out_d[:, (y0 + tr) * W : (y0 + tr + 8) * W], in_=osb
        )

    return tail


def build_module():
    nc = bacc.Bacc(
        "TRN2",
        target_bir_lowering=False,
        debug=False,
        enable_asserts=False,
        num_devices=B,
    )
    d = {
        "xpad": nc.dram_tensor("xpad", [C, HP * WP], BF16, kind="ExternalInput").ap(),
        "wblob_a": nc.dram_tensor("wblob_a", [128, WA_COLS], BF16, kind="ExternalInput").ap(),
        "wblob_b": nc.dram_tensor("wblob_b", [128, WB_COLS], BF16, kind="ExternalInput").ap(),
        "cb": nc.dram_tensor("cb", [C, 1], F32, kind="ExternalInput").ap(),
        "mask": nc.dram_tensor("mask", [128, NT * 128], BF16, kind="ExternalInput").ap(),
        "out": nc.dram_tensor("out", [C, HW], BF16, kind="ExternalOutput").ap(),
    }
    with tile.TileContext(nc) as tc:
        with ExitStack() as ctx:
            d["ctx"] = ctx
            _emit(tc, nc, d)
    nc.finalize()
    return nc


_module_cache = None


def _get_module():
    global _module_cache
    if _module_cache is None:
        _module_cache = build_module()
    return _module_cache


def make_in_maps(x, altitude, W1, W2, conv_w, conv_b, ca_w1, ca_w2):
    f = np.float32
    bf = ml_dtypes.bfloat16
    x = np.asarray(x, dtype=f)
    altitude = np.asarray(altitude, dtype=f)
    xpad = np.zeros((B, C, HP, WP), dtype=f)
    xpad[:, :, 1 : H + 1, XOFF : XOFF + W] = x
    xq = np.ascontiguousarray(xpad.astype(bf).reshape(B, C, HP * WP))

    wblob_shared = np.zeros((128, WA_COLS), dtype=bf)
    wblob_shared[:, W_W1T : W_W1T + 128] = np.asarray(W1, dtype=f).T.astype(bf)
    wblob_shared[:, W_CA1T : W_CA1T + 16] = np.asarray(ca_w1, dtype=f).T.astype(bf)
    wblob_shared[0:16, W_CA2T : W_CA2T + 128] = np.asarray(
        ca_w2, dtype=f
    ).T.astype(bf)
    wblob_b = np.zeros((128, WB_COLS), dtype=bf)
    wblob_b[:, W_W2T : W_W2T + NT * 128] = np.asarray(W2, dtype=f).T.astype(bf)
    wblob_b[:, W_CWT : W_CWT + 128] = np.asarray(conv_w, dtype=f).T.astype(bf)
    wblob_b = np.ascontiguousarray(wblob_b)

    cb_arr = np.ascontiguousarray(np.asarray(conv_b, dtype=f).reshape(C, 1))
    mask_arr = np.ascontiguousarray(
        np.tile(np.eye(128, dtype=f), (1, NT)).astype(bf)
    )

    maps = []
    for bb in range(B):
        wblob_a = wblob_shared.copy()
        wblob_a[:, W_ALT] = altitude[bb].astype(bf)
        maps.append({"xpad": xq[bb], "wblob_a": np.ascontiguousarray(wblob_a),
                     "wblob_b": wblob_b, "cb": cb_arr, "mask": mask_arr})
    return maps


def kernel(x, altitude, W1, W2, conv_w, conv_b, ca_w1, ca_w2):
    global last_results
    in_maps = make_in_maps(x, altitude, W1, W2, conv_w, conv_b, ca_w1, ca_w2)
    nc = _get_module()
    trace = os.environ.get("KERNEL_TRACE", "0") == "1"
    last_results = run_bass_kernel_spmd(
        nc, in_maps, core_ids=list(range(B)), trace=trace
    )
    out = np.stack(
        [
            last_results.results[bb]["out"].astype(np.float32).reshape(C, H, W)
            for bb in range(B)
        ]
    )
    return out
